# revision 2
# baseline (speedup 1.0000x reference)
"""Self-contained Trainium2 (Bass/Tile) kernel for the causal-attention module.

Problem shapes (hardcoded): x [2, 2048, 2048] fp32, rotary_emb [2048, 64] fp32,
gamma [2048] fp32, Wq [2048, 2048], Wkv [2048, 4096], Wout [2048, 2048] fp32.

Sharding: 8 NeuronCores = 2 batches (data parallel) x 4 head groups of 8 heads
(tensor parallel).  Each core computes a full [2048, 2048] partial output
(its head group's contribution through Wout's row block); the host sums the
4 partials per batch.

Per-core kernel: transpose-free attention.
  - x is cast to bf16 and transposed on-chip (DMA xbar) to xT [c, i].
  - Q^T/K^T are projected directly into d-major layout [hd, i]; V in natural
    [j, hd] layout with an appended ones column (softmax denominator).
  - Scores are computed transposed S^T[j, i]; exp on ScalarE (no running max
    needed: |scores| <= ~10 with these operand scales); causal handled by
    skipping fully-masked blocks, restricting partially-masked matmul column
    ranges, and a triangular 0/1 mask multiply on diagonal 128x128 blocks.
  - AV matmul consumes P^T directly with V natural; the ones column row gives
    the denominator, inverted and broadcast via a rank-1 matmul.
  - RMSNorm is folded: gamma into host-side weights; the per-token scale
    r = sqrt(DIM)/||x_i|| into the rotary tables (covers Q and K) and the V
    store (per-partition scalar).
All matmul operands bf16, accumulation fp32 in PSUM, output fp32.
"""

from contextlib import ExitStack

import numpy as np
import ml_dtypes

B, N, DIM = 2, 2048, 2048
HEADS_TOTAL, DH = 32, 64
N_CORES = 8
GROUPS = 4
HEADS = HEADS_TOTAL // GROUPS      # heads per core
HD = HEADS * DH                    # 512
IB = 512                           # query i-block width

_CACHED = {}


def _build():
    import concourse.tile as tile
    from concourse import mybir, bacc

    F32 = mybir.dt.float32
    BF16 = mybir.dt.bfloat16
    AF = mybir.ActivationFunctionType
    ALU = mybir.AluOpType

    NT = N
    n_tt = NT // 128
    n_ct = DIM // 128
    n_ib = NT // IB
    n_hb = HD // 128
    jpi = IB // 128

    nc = bacc.Bacc(None)
    x_d = nc.declare_dram_parameter("x", [NT, DIM], F32, isOutput=False)
    wq_d = nc.declare_dram_parameter("wq", [DIM, HD], BF16, isOutput=False)
    wk_d = nc.declare_dram_parameter("wk", [DIM, HD], BF16, isOutput=False)
    wv_d = nc.declare_dram_parameter("wv", [DIM, HD], BF16, isOutput=False)
    wout_d = nc.declare_dram_parameter("wout", [HD, DIM], BF16, isOutput=False)
    cosr_d = nc.declare_dram_parameter("cosr", [128, NT], BF16, isOutput=False)
    sinr_d = nc.declare_dram_parameter("sinr", [128, NT], BF16, isOutput=False)
    tri_d = nc.declare_dram_parameter("tri", [128, 128], BF16, isOutput=False)
    out_d = nc.declare_dram_parameter("out", [NT, DIM], F32, isOutput=True)

    ctx = ExitStack()
    with ctx:
        tc = ctx.enter_context(tile.TileContext(nc))
        pers = ctx.enter_context(tc.tile_pool(name="pers", bufs=1))
        trans = ctx.enter_context(tc.tile_pool(name="trans", bufs=1))
        xpool = ctx.enter_context(tc.tile_pool(name="xin", bufs=2))
        wpool = ctx.enter_context(tc.tile_pool(name="wqk", bufs=1))
        epool = ctx.enter_context(tc.tile_pool(name="exp", bufs=2))
        opool = ctx.enter_context(tc.tile_pool(name="ostage", bufs=2))
        ps = ctx.enter_context(tc.tile_pool(name="ps", bufs=2, space="PSUM"))
        ps_sc = ctx.enter_context(tc.tile_pool(name="pssc", bufs=2, space="PSUM"))
        ps_av = ctx.enter_context(tc.tile_pool(name="psav", bufs=1, space="PSUM"))

        xT = [pers.tile([128, NT], BF16, tag=f"xT{c}", name=f"xT{c}")
              for c in range(n_ct)]
        qt = [pers.tile([128, NT], BF16, tag=f"qt{h}", name=f"qt{h}")
              for h in range(n_hb)]
        kt = [pers.tile([128, NT], BF16, tag=f"kt{h}", name=f"kt{h}")
              for h in range(n_hb)]
        vst = [pers.tile([128, HEADS, DH + 1], BF16, tag=f"v{t}", name=f"v{t}")
               for t in range(n_tt)]
        wv_sb = pers.tile([128, n_ct, HD], BF16, tag="wv")
        wout_sb = pers.tile([128, n_hb, DIM], BF16, tag="wout")
        crep = pers.tile([128, NT], BF16, tag="crep")
        srep = pers.tile([128, NT], BF16, tag="srep")
        tri = pers.tile([128, 128], BF16, tag="tri")
        ss = pers.tile([128, n_tt], F32, tag="ss")
        rt = pers.tile([128, n_tt], F32, tag="rt")
        r_row = pers.tile([1, NT], BF16, tag="r_row")
        ones_f = pers.tile([128, 128], F32, tag="ones_f")
        ones_b = pers.tile([1, 128], BF16, tag="ones_b")

        nc.sync.dma_start(crep[:], cosr_d[:])   # raw cos; r folded in below
        nc.sync.dma_start(srep[:], sinr_d[:])
        nc.sync.dma_start(tri[:], tri_d[:])
        nc.vector.memset(ones_f[:], 1.0)
        nc.vector.memset(ones_b[:], 1.0)
        nc.sync.dma_start(wv_sb[:], wv_d.rearrange("(c p) h -> p c h", p=128))
        nc.sync.dma_start(wout_sb[:], wout_d.rearrange("(g p) e -> p g e", p=128))

        # phase 0: load x (cast bf16), sum of squares, on-chip transpose
        for t in range(n_tt):
            xb = xpool.tile([128, DIM], BF16, tag="xb", name="xb")
            nc.gpsimd.dma_start(out=xb[:], in_=x_d[t * 128:(t + 1) * 128, :])
            junk = xpool.tile([128, DIM], BF16, tag="junk", name="junk", bufs=1)
            nc.scalar.activation(out=junk[:], in_=xb[:], func=AF.Square,
                                 accum_out=ss[:, t:t + 1])
            for c in range(n_ct):
                nc.sync.dma_start(
                    out=xT[c][:, t * 128:(t + 1) * 128],
                    in_=xb[:, c * 128:(c + 1) * 128], transpose=True)

        # r = sqrt(DIM)/||x_i||, one Newton polish of the rsqrt
        m_sc = 1.0 / DIM
        nc.scalar.activation(out=rt[:], in_=ss[:], func=AF.Sqrt, scale=m_sc)
        nc.vector.reciprocal(out=rt[:], in_=rt[:])
        t1 = trans.tile([128, n_tt], F32, tag="nt1", name="nt1")
        nc.vector.tensor_mul(out=t1[:], in0=rt[:], in1=rt[:])
        nc.vector.tensor_mul(out=t1[:], in0=t1[:], in1=ss[:])
        nc.vector.tensor_scalar(out=t1[:], in0=t1[:], scalar1=-0.5 * m_sc,
                                scalar2=1.5, op0=ALU.mult, op1=ALU.add)
        nc.vector.tensor_mul(out=rt[:], in0=rt[:], in1=t1[:])
        for t in range(n_tt):
            nc.gpsimd.dma_start(out=r_row[:, t * 128:(t + 1) * 128],
                                in_=rt[:, t:t + 1])
        for i in range(NT // 512):
            sl = slice(i * 512, (i + 1) * 512)
            pbc = ps.tile([128, 512], F32, tag="proj", name="pbc")
            nc.tensor.matmul(pbc[:], lhsT=ones_b[:], rhs=r_row[:, sl],
                             start=True, stop=True)
            nc.vector.tensor_mul(out=crep[:, sl], in0=pbc[:], in1=crep[:, sl])
            nc.vector.tensor_mul(out=srep[:, sl], in0=pbc[:], in1=srep[:, sl])

        # phase 1a: V projection + r scale + ones column
        for t in range(n_tt):
            psv = ps.tile([128, HD], F32, tag="proj", name="psv")
            for c in range(n_ct):
                nc.tensor.matmul(psv[:], lhsT=xT[c][:, t * 128:(t + 1) * 128],
                                 rhs=wv_sb[:, c, :], start=(c == 0),
                                 stop=(c == n_ct - 1))
            nc.vector.tensor_scalar_mul(
                out=vst[t][:, :, 0:DH],
                in0=psv[:].rearrange("p (h d) -> p h d", h=HEADS),
                scalar1=rt[:, t:t + 1])
            nc.vector.memset(vst[t][:, :, DH:DH + 1], 1.0)

        # phase 1b: Q/K projections into d-major + rotary (r folded)
        for (w_d, dst) in ((wq_d, qt), (wk_d, kt)):
            for h in range(n_hb):
                w_sb = wpool.tile([128, n_ct, 128], BF16, tag="w", name="w_sb")
                nc.sync.dma_start(
                    w_sb[:],
                    w_d[:, h * 128:(h + 1) * 128].rearrange("(c p) m -> p c m",
                                                            p=128))
                for i in range(n_ib):
                    sl = slice(i * IB, (i + 1) * IB)
                    psq = ps.tile([128, IB], F32, tag="proj", name="psq")
                    for c in range(n_ct):
                        nc.tensor.matmul(psq[:], lhsT=w_sb[:, c, :],
                                         rhs=xT[c][:, sl], start=(c == 0),
                                         stop=(c == n_ct - 1))
                    nc.any.tensor_copy(dst[h][:, sl], psq[:])
                # rotate_half partition swap: [32:64, 0:32, 96:128, 64:96]
                swp = trans.tile([128, NT], BF16, tag="swpqk", name="swp")
                nc.sync.dma_start(out=swp[0:32, :], in_=dst[h][32:64, :])
                nc.sync.dma_start(out=swp[32:64, :], in_=dst[h][0:32, :])
                nc.sync.dma_start(out=swp[64:96, :], in_=dst[h][96:128, :])
                nc.sync.dma_start(out=swp[96:128, :], in_=dst[h][64:96, :])
                tmp = trans.tile([128, NT], BF16, tag="tmpqk", name="tmp")
                nc.vector.tensor_mul(out=tmp[:], in0=dst[h][:], in1=crep[:])
                nc.vector.tensor_mul(out=dst[h][:], in0=swp[:], in1=srep[:])
                nc.vector.tensor_add(out=dst[h][:], in0=dst[h][:], in1=tmp[:])

        # phase 2: attention per (i-block, head pair)
        # ont0/1 reuse the (now dead) rotary scratch slots; ont2/3 are fresh
        ont = [trans.tile([128, NT], BF16, tag="swpqk", name="ont0"),
               trans.tile([128, NT], BF16, tag="tmpqk", name="ont1"),
               pers.tile([128, NT], BF16, tag="ont2", name="ont2"),
               pers.tile([128, NT], BF16, tag="ont3", name="ont3")]
        scale = DH ** -0.5
        for i in range(n_ib):
            n_jb = jpi * i + jpi
            isl = slice(i * IB, (i + 1) * IB)
            for hp in range(n_hb):
                o_ps = [ps_av.tile([DH + 1, IB], F32, tag=f"oav{k}",
                                   name=f"oav{k}") for k in (0, 1)]
                for jb in range(n_jb):
                    delta = jb - jpi * i
                    v0 = max(delta, 0) * 128
                    vsl = slice(i * IB + v0, (i + 1) * IB)
                    psl = slice(v0, IB)
                    jsl = slice(jb * 128, (jb + 1) * 128)
                    e_t = []
                    for k in (0, 1):
                        hsl = slice(k * 64, (k + 1) * 64)
                        s_ps = ps_sc.tile([128, IB], F32, tag=f"sc{k}",
                                          name=f"sc{k}")
                        nc.tensor.matmul(s_ps[:, psl], lhsT=kt[hp][hsl, jsl],
                                         rhs=qt[hp][hsl, vsl], start=True,
                                         stop=True)
                        e = epool.tile([128, IB], BF16, tag=f"e{k}", name=f"e{k}")
                        nc.scalar.activation(out=e[:, psl], in_=s_ps[:, psl],
                                             func=AF.Exp, scale=scale)
                        if delta >= 0:
                            dsl = slice(v0, v0 + 128)
                            nc.vector.tensor_mul(out=e[:, dsl], in0=e[:, dsl],
                                                 in1=tri[:])
                        e_t.append(e)
                    for k in (0, 1):
                        h = hp * 2 + k
                        nc.tensor.matmul(o_ps[k][:, psl], lhsT=vst[jb][:, h, :],
                                         rhs=e_t[k][:, psl], start=(jb == 0),
                                         stop=(jb == n_jb - 1))
                rec = trans.tile([128, 2 * IB], F32, tag="rec", name="rec")
                bc_sb = trans.tile([128, 2 * IB], BF16, tag="bc", name="bc_sb")
                for k in (0, 1):
                    bsl = slice(k * IB, (k + 1) * IB)
                    nc.vector.reciprocal(out=rec[64:65, bsl],
                                         in_=o_ps[k][DH:DH + 1, :])
                    pbc2 = ps.tile([128, IB], F32, tag="proj", name="pbc2")
                    nc.tensor.matmul(pbc2[:], lhsT=ones_f[64:65, :],
                                     rhs=rec[64:65, bsl], start=True, stop=True)
                    nc.scalar.copy(bc_sb[:, bsl], pbc2[:])
                nc.vector.tensor_mul(out=ont[hp][0:64, isl], in0=o_ps[0][0:DH, :],
                                     in1=bc_sb[0:64, 0:IB])
                nc.vector.tensor_mul(out=ont[hp][64:128, isl],
                                     in0=o_ps[1][0:DH, :],
                                     in1=bc_sb[64:128, IB:2 * IB])

            # phase 3: output projection for this i-block
            for ic in range(IB // 128):
                r0 = i * IB + ic * 128
                for ec in range(DIM // 512):
                    pso = ps.tile([128, 512], F32, tag="proj", name="pso")
                    for hp in range(n_hb):
                        nc.tensor.matmul(
                            pso[:], lhsT=ont[hp][:, r0:r0 + 128],
                            rhs=wout_sb[:, hp, ec * 512:(ec + 1) * 512],
                            start=(hp == 0), stop=(hp == n_hb - 1))
                    oc = opool.tile([128, 512], F32, tag="oc", name="oc")
                    nc.any.tensor_copy(oc[:], pso[:])
                    nc.sync.dma_start(
                        out=out_d[r0:r0 + 128, ec * 512:(ec + 1) * 512],
                        in_=oc[:])

    nc.compile()
    return nc


def get_nc():
    if "nc" not in _CACHED:
        _CACHED["nc"] = _build()
    return _CACHED["nc"]


def host_inputs(x, rotary_emb, gamma, Wq, Wkv, Wout):
    """Build the 8 per-core input dicts."""
    bf = ml_dtypes.bfloat16
    x = np.asarray(x, np.float32)
    gW = np.asarray(gamma, np.float32)[:, None]
    Wq = gW * np.asarray(Wq, np.float32)
    Wkv = np.asarray(Wkv, np.float32)
    Wk = gW * Wkv[:, :HEADS_TOTAL * DH]
    Wv = gW * Wkv[:, HEADS_TOTAL * DH:]
    Wout = np.asarray(Wout, np.float32)
    pos = np.asarray(rotary_emb, np.float32)
    cos = np.cos(pos).T
    sgn = np.concatenate([-np.ones(DH // 2), np.ones(DH // 2)]).astype(np.float32)
    sin = (np.sin(pos) * sgn[None, :]).T
    cosr = np.ascontiguousarray(np.tile(cos, (2, 1)).astype(bf))
    sinr = np.ascontiguousarray(np.tile(sin, (2, 1)).astype(bf))
    jj, ii = np.mgrid[0:128, 0:128]
    tri = np.ascontiguousarray((jj <= ii).astype(bf))
    maps = []
    for core in range(N_CORES):
        b, g = core // GROUPS, core % GROUPS
        hs = slice(g * HD, (g + 1) * HD)
        maps.append({
            "x": np.ascontiguousarray(x[b]),
            "wq": np.ascontiguousarray(Wq[:, hs].astype(bf)),
            "wk": np.ascontiguousarray(Wk[:, hs].astype(bf)),
            "wv": np.ascontiguousarray(Wv[:, hs].astype(bf)),
            "wout": np.ascontiguousarray(Wout[hs, :].astype(bf)),
            "cosr": cosr, "sinr": sinr, "tri": tri,
        })
    return maps


def run_cores(in_maps, trace=False, **kwargs):
    from concourse.bass_utils import run_bass_kernel_spmd
    nc = get_nc()
    return run_bass_kernel_spmd(nc, in_maps, list(range(N_CORES)), trace=trace,
                                **kwargs)


def kernel(x, rotary_emb, gamma, Wq, Wkv, Wout):
    in_maps = host_inputs(x, rotary_emb, gamma, Wq, Wkv, Wout)
    res = run_cores(in_maps, trace=False)
    out = np.zeros((B, N, DIM), np.float32)
    for core in range(N_CORES):
        b = core // GROUPS
        out[b] += res.results[core]["out"]
    return out


# revision 9
# speedup vs baseline: 1.4184x; 1.4184x over previous
"""Self-contained Trainium2 (Bass/Tile) kernel for the causal-attention module.

Problem shapes (hardcoded): x [2, 2048, 2048] fp32, rotary_emb [2048, 64] fp32,
gamma [2048] fp32, Wq [2048, 2048], Wkv [2048, 4096], Wout [2048, 2048] fp32.

Sharding: 8 NeuronCores = 2 batches (data parallel) x 4 head groups of 8 heads
(tensor parallel).  Each core computes a full [2048, 2048] partial output
(its head group's contribution through Wout's row block); the host sums the
4 partials per batch.

Per-core kernel: transpose-free attention.
  - x is cast to bf16 and transposed on-chip (DMA xbar) to xT [c, i].
  - Q^T/K^T are projected directly into d-major layout [hd, i]; V in natural
    [j, hd] layout with an appended ones column (softmax denominator).
  - Scores are computed transposed S^T[j, i]; exp on ScalarE (no running max
    needed: |scores| <= ~10 with these operand scales); causal handled by
    skipping fully-masked blocks, restricting partially-masked matmul column
    ranges, and a triangular 0/1 mask multiply on diagonal 128x128 blocks.
  - AV matmul consumes P^T directly with V natural; the ones column row gives
    the denominator, inverted and broadcast via a rank-1 matmul.
  - RMSNorm is folded: gamma into host-side weights; the per-token scale
    r = sqrt(DIM)/||x_i|| into the rotary tables (covers Q and K) and the V
    store (per-partition scalar).
All matmul operands bf16, accumulation fp32 in PSUM, output fp32.
"""

from contextlib import ExitStack

import numpy as np
import ml_dtypes

B, N, DIM = 2, 2048, 2048
HEADS_TOTAL, DH = 32, 64
N_CORES = 8
GROUPS = 4
HEADS = HEADS_TOTAL // GROUPS      # heads per core
HD = HEADS * DH                    # 512
IB = 512                           # query i-block width

_CACHED = {}


def _build():
    import concourse.tile as tile
    from concourse import mybir, bacc

    F32 = mybir.dt.float32
    BF16 = mybir.dt.bfloat16
    AF = mybir.ActivationFunctionType
    ALU = mybir.AluOpType

    NT = N
    n_tt = NT // 128
    n_ct = DIM // 128
    n_ib = NT // IB
    n_hb = HD // 128
    jpi = IB // 128

    nc = bacc.Bacc(None)
    x_d = nc.declare_dram_parameter("x", [NT, DIM], F32, isOutput=False)
    wq_d = nc.declare_dram_parameter("wq", [DIM, HD], BF16, isOutput=False)
    wk_d = nc.declare_dram_parameter("wk", [DIM, HD], BF16, isOutput=False)
    wv_d = nc.declare_dram_parameter("wv", [DIM, HD], BF16, isOutput=False)
    wout_d = nc.declare_dram_parameter("wout", [HD, DIM], BF16, isOutput=False)
    cosr_d = nc.declare_dram_parameter("cosr", [128, NT], BF16, isOutput=False)
    sinr_d = nc.declare_dram_parameter("sinr", [128, NT], BF16, isOutput=False)
    tri_d = nc.declare_dram_parameter("tri", [128, 128], BF16, isOutput=False)
    ident_d = nc.declare_dram_parameter("ident", [128, 128], BF16, isOutput=False)
    out_d = nc.declare_dram_parameter("out", [NT, DIM], F32, isOutput=True)

    ctx = ExitStack()
    with ctx:
        tc = ctx.enter_context(tile.TileContext(nc))
        pers = ctx.enter_context(tc.tile_pool(name="pers", bufs=1))
        trans = ctx.enter_context(tc.tile_pool(name="trans", bufs=1))
        xpool = ctx.enter_context(tc.tile_pool(name="xin", bufs=2))
        wpool = ctx.enter_context(tc.tile_pool(name="wqk", bufs=1))
        epool = ctx.enter_context(tc.tile_pool(name="exp", bufs=2))
        opool = ctx.enter_context(tc.tile_pool(name="ostage", bufs=2))
        ps = ctx.enter_context(tc.tile_pool(name="ps", bufs=2, space="PSUM"))
        ps_sc = ctx.enter_context(tc.tile_pool(name="pssc", bufs=2, space="PSUM"))
        ps_av = ctx.enter_context(tc.tile_pool(name="psav", bufs=1, space="PSUM"))

        xT = [pers.tile([128, NT], BF16, tag=f"xT{c}", name=f"xT{c}")
              for c in range(n_ct)]
        qt = [pers.tile([128, NT], BF16, tag=f"qt{h}", name=f"qt{h}")
              for h in range(n_hb)]
        kt = [pers.tile([128, NT], BF16, tag=f"kt{h}", name=f"kt{h}")
              for h in range(n_hb)]
        vst = [pers.tile([128, HEADS, DH + 1], BF16, tag=f"v{t}", name=f"v{t}")
               for t in range(n_tt)]
        wv_sb = pers.tile([128, n_ct, HD], BF16, tag="wv")
        wout_sb = pers.tile([128, n_hb, DIM], BF16, tag="wout")
        crep = pers.tile([128, NT], BF16, tag="crep")
        srep = pers.tile([128, NT], BF16, tag="srep")
        tri = pers.tile([128, 128], BF16, tag="tri")
        ss = pers.tile([128, n_tt], F32, tag="ss")
        rt = pers.tile([128, n_tt], F32, tag="rt")
        r_row = pers.tile([1, NT], BF16, tag="r_row")
        ones_f = pers.tile([128, 128], F32, tag="ones_f")
        ones_b = pers.tile([1, 128], BF16, tag="ones_b")

        nc.sync.dma_start(crep[:], cosr_d[:])   # raw cos; r folded in below
        nc.sync.dma_start(srep[:], sinr_d[:])
        nc.sync.dma_start(tri[:], tri_d[:])
        ident = pers.tile([128, 128], BF16, tag="ident")
        nc.sync.dma_start(ident[:], ident_d[:])
        nc.vector.memset(ones_f[:], 1.0)
        nc.vector.memset(ones_b[:], 1.0)
        nc.sync.dma_start(wv_sb[:], wv_d.rearrange("(c p) h -> p c h", p=128))
        nc.sync.dma_start(wout_sb[:], wout_d.rearrange("(g p) e -> p g e", p=128))

        # phase 0: load x (cast bf16), sum of squares, on-chip transpose.
        # DMA-xbar transposes occupy a HWDGE sequencer ~1.2us EACH and Tile
        # serializes them globally, so transpose on the (otherwise idle)
        # TensorEngine instead, with DVE draining PSUM->SBUF.
        for t in range(n_tt):
            xb = xpool.tile([128, DIM], BF16, tag="xb", name="xb")
            nc.gpsimd.dma_start(out=xb[:], in_=x_d[t * 128:(t + 1) * 128, :])
            junk = xpool.tile([128, DIM], BF16, tag="junk", name="junk", bufs=1)
            nc.scalar.activation(out=junk[:], in_=xb[:], func=AF.Square,
                                 accum_out=ss[:, t:t + 1])
            for c in range(n_ct):
                pt = ps.tile([128, 128], BF16, tag="proj", name="pt")
                nc.tensor.transpose(pt[:], xb[:, c * 128:(c + 1) * 128], ident[:])
                nc.vector.tensor_copy(xT[c][:, t * 128:(t + 1) * 128], pt[:])

        # r = sqrt(DIM)/||x_i||, one Newton polish of the rsqrt
        m_sc = 1.0 / DIM
        nc.scalar.activation(out=rt[:], in_=ss[:], func=AF.Sqrt, scale=m_sc)
        nc.vector.reciprocal(out=rt[:], in_=rt[:])
        t1 = trans.tile([128, n_tt], F32, tag="nt1", name="nt1")
        nc.vector.tensor_mul(out=t1[:], in0=rt[:], in1=rt[:])
        nc.vector.tensor_mul(out=t1[:], in0=t1[:], in1=ss[:])
        nc.vector.tensor_scalar(out=t1[:], in0=t1[:], scalar1=-0.5 * m_sc,
                                scalar2=1.5, op0=ALU.mult, op1=ALU.add)
        nc.vector.tensor_mul(out=rt[:], in0=rt[:], in1=t1[:])
        for t in range(n_tt):
            nc.gpsimd.dma_start(out=r_row[:, t * 128:(t + 1) * 128],
                                in_=rt[:, t:t + 1])
        for i in range(NT // 512):
            sl = slice(i * 512, (i + 1) * 512)
            pbc = ps.tile([128, 512], F32, tag="proj", name="pbc")
            nc.tensor.matmul(pbc[:], lhsT=ones_b[:], rhs=r_row[:, sl],
                             start=True, stop=True)
            nc.vector.tensor_mul(out=crep[:, sl], in0=pbc[:], in1=crep[:, sl])
            nc.vector.tensor_mul(out=srep[:, sl], in0=pbc[:], in1=srep[:, sl])

        # phase 1a: V projection + r scale + ones column
        for t in range(n_tt):
            psv = ps.tile([128, HD], F32, tag="proj", name="psv")
            for c in range(n_ct):
                nc.tensor.matmul(psv[:], lhsT=xT[c][:, t * 128:(t + 1) * 128],
                                 rhs=wv_sb[:, c, :], start=(c == 0),
                                 stop=(c == n_ct - 1))
            nc.vector.tensor_scalar_mul(
                out=vst[t][:, :, 0:DH],
                in0=psv[:].rearrange("p (h d) -> p h d", h=HEADS),
                scalar1=rt[:, t:t + 1])
            nc.vector.memset(vst[t][:, :, DH:DH + 1], 1.0)

        # phase 1b: Q/K projections into d-major + rotary (r folded)
        for (w_d, dst) in ((wq_d, qt), (wk_d, kt)):
            for h in range(n_hb):
                w_sb = wpool.tile([128, n_ct, 128], BF16, tag="w", name="w_sb")
                nc.sync.dma_start(
                    w_sb[:],
                    w_d[:, h * 128:(h + 1) * 128].rearrange("(c p) m -> p c m",
                                                            p=128))
                for i in range(n_ib):
                    sl = slice(i * IB, (i + 1) * IB)
                    psq = ps.tile([128, IB], F32, tag="proj", name="psq")
                    for c in range(n_ct):
                        nc.tensor.matmul(psq[:], lhsT=w_sb[:, c, :],
                                         rhs=xT[c][:, sl], start=(c == 0),
                                         stop=(c == n_ct - 1))
                    nc.any.tensor_copy(dst[h][:, sl], psq[:])
                # rotate_half partition swap: [32:64, 0:32, 96:128, 64:96]
                swp = trans.tile([128, NT], BF16, tag="swpqk", name="swp")
                nc.sync.dma_start(out=swp[0:32, :], in_=dst[h][32:64, :])
                nc.sync.dma_start(out=swp[32:64, :], in_=dst[h][0:32, :])
                nc.sync.dma_start(out=swp[64:96, :], in_=dst[h][96:128, :])
                nc.sync.dma_start(out=swp[96:128, :], in_=dst[h][64:96, :])
                tmp = trans.tile([128, NT], BF16, tag="tmpqk", name="tmp")
                nc.vector.tensor_mul(out=tmp[:], in0=dst[h][:], in1=crep[:])
                nc.vector.tensor_mul(out=dst[h][:], in0=swp[:], in1=srep[:])
                nc.vector.tensor_add(out=dst[h][:], in0=dst[h][:], in1=tmp[:])

        # phase 2: attention per (i-block, head pair)
        # ont0/1 reuse the (now dead) rotary scratch slots; ont2/3 are fresh
        ont = [trans.tile([128, NT], BF16, tag="swpqk", name="ont0"),
               trans.tile([128, NT], BF16, tag="tmpqk", name="ont1"),
               pers.tile([128, NT], BF16, tag="ont2", name="ont2"),
               pers.tile([128, NT], BF16, tag="ont3", name="ont3")]
        scale = DH ** -0.5
        for i in range(n_ib):
            n_jb = jpi * i + jpi
            isl = slice(i * IB, (i + 1) * IB)
            for hp in range(n_hb):
                o_ps = [ps_av.tile([DH + 1, IB], F32, tag=f"oav{k}",
                                   name=f"oav{k}") for k in (0, 1)]
                for jb in range(n_jb):
                    delta = jb - jpi * i
                    v0 = max(delta, 0) * 128
                    vsl = slice(i * IB + v0, (i + 1) * IB)
                    psl = slice(v0, IB)
                    jsl = slice(jb * 128, (jb + 1) * 128)
                    e_t = []
                    for k in (0, 1):
                        hsl = slice(k * 64, (k + 1) * 64)
                        s_ps = ps_sc.tile([128, IB], F32, tag=f"sc{k}",
                                          name=f"sc{k}")
                        nc.tensor.matmul(s_ps[:, psl], lhsT=kt[hp][hsl, jsl],
                                         rhs=qt[hp][hsl, vsl], start=True,
                                         stop=True)
                        e = epool.tile([128, IB], BF16, tag=f"e{k}", name=f"e{k}")
                        nc.scalar.activation(out=e[:, psl], in_=s_ps[:, psl],
                                             func=AF.Exp, scale=scale)
                        if delta >= 0:
                            dsl = slice(v0, v0 + 128)
                            nc.vector.tensor_mul(out=e[:, dsl], in0=e[:, dsl],
                                                 in1=tri[:])
                        e_t.append(e)
                    for k in (0, 1):
                        h = hp * 2 + k
                        nc.tensor.matmul(o_ps[k][:, psl], lhsT=vst[jb][:, h, :],
                                         rhs=e_t[k][:, psl], start=(jb == 0),
                                         stop=(jb == n_jb - 1))
                rec = trans.tile([128, 2 * IB], F32, tag="rec", name="rec")
                bc_sb = trans.tile([128, 2 * IB], BF16, tag="bc", name="bc_sb")
                for k in (0, 1):
                    bsl = slice(k * IB, (k + 1) * IB)
                    nc.vector.reciprocal(out=rec[64:65, bsl],
                                         in_=o_ps[k][DH:DH + 1, :])
                    pbc2 = ps.tile([128, IB], F32, tag="proj", name="pbc2")
                    nc.tensor.matmul(pbc2[:], lhsT=ones_f[64:65, :],
                                     rhs=rec[64:65, bsl], start=True, stop=True)
                    nc.scalar.copy(bc_sb[:, bsl], pbc2[:])
                nc.vector.tensor_mul(out=ont[hp][0:64, isl], in0=o_ps[0][0:DH, :],
                                     in1=bc_sb[0:64, 0:IB])
                nc.vector.tensor_mul(out=ont[hp][64:128, isl],
                                     in0=o_ps[1][0:DH, :],
                                     in1=bc_sb[64:128, IB:2 * IB])

            # phase 3: output projection for this i-block
            for ic in range(IB // 128):
                r0 = i * IB + ic * 128
                for ec in range(DIM // 512):
                    pso = ps.tile([128, 512], F32, tag="proj", name="pso")
                    for hp in range(n_hb):
                        nc.tensor.matmul(
                            pso[:], lhsT=ont[hp][:, r0:r0 + 128],
                            rhs=wout_sb[:, hp, ec * 512:(ec + 1) * 512],
                            start=(hp == 0), stop=(hp == n_hb - 1))
                    oc = opool.tile([128, 512], F32, tag="oc", name="oc")
                    nc.any.tensor_copy(oc[:], pso[:])
                    nc.sync.dma_start(
                        out=out_d[r0:r0 + 128, ec * 512:(ec + 1) * 512],
                        in_=oc[:])

    nc.compile()
    return nc


def get_nc():
    if "nc" not in _CACHED:
        _CACHED["nc"] = _build()
    return _CACHED["nc"]


def host_inputs(x, rotary_emb, gamma, Wq, Wkv, Wout):
    """Build the 8 per-core input dicts."""
    bf = ml_dtypes.bfloat16
    x = np.asarray(x, np.float32)
    gW = np.asarray(gamma, np.float32)[:, None]
    Wq = gW * np.asarray(Wq, np.float32)
    Wkv = np.asarray(Wkv, np.float32)
    Wk = gW * Wkv[:, :HEADS_TOTAL * DH]
    Wv = gW * Wkv[:, HEADS_TOTAL * DH:]
    Wout = np.asarray(Wout, np.float32)
    pos = np.asarray(rotary_emb, np.float32)
    cos = np.cos(pos).T
    sgn = np.concatenate([-np.ones(DH // 2), np.ones(DH // 2)]).astype(np.float32)
    sin = (np.sin(pos) * sgn[None, :]).T
    cosr = np.ascontiguousarray(np.tile(cos, (2, 1)).astype(bf))
    sinr = np.ascontiguousarray(np.tile(sin, (2, 1)).astype(bf))
    jj, ii = np.mgrid[0:128, 0:128]
    tri = np.ascontiguousarray((jj <= ii).astype(bf))
    ident = np.ascontiguousarray(np.eye(128, dtype=np.float32).astype(bf))
    maps = []
    for core in range(N_CORES):
        b, g = core // GROUPS, core % GROUPS
        hs = slice(g * HD, (g + 1) * HD)
        maps.append({
            "x": np.ascontiguousarray(x[b]),
            "wq": np.ascontiguousarray(Wq[:, hs].astype(bf)),
            "wk": np.ascontiguousarray(Wk[:, hs].astype(bf)),
            "wv": np.ascontiguousarray(Wv[:, hs].astype(bf)),
            "wout": np.ascontiguousarray(Wout[hs, :].astype(bf)),
            "cosr": cosr, "sinr": sinr, "tri": tri, "ident": ident,
        })
    return maps


def run_cores(in_maps, trace=False, **kwargs):
    from concourse.bass_utils import run_bass_kernel_spmd
    nc = get_nc()
    return run_bass_kernel_spmd(nc, in_maps, list(range(N_CORES)), trace=trace,
                                **kwargs)


def kernel(x, rotary_emb, gamma, Wq, Wkv, Wout):
    in_maps = host_inputs(x, rotary_emb, gamma, Wq, Wkv, Wout)
    res = run_cores(in_maps, trace=False)
    out = np.zeros((B, N, DIM), np.float32)
    for core in range(N_CORES):
        b = core // GROUPS
        out[b] += res.results[core]["out"]
    return out
